# revision 9
# baseline (speedup 1.0000x reference)
"""FedGATConv forward kernel for Trainium2 (Bass/Tile), 8-core data-parallel.

Computation per node n (N=4096, F=128, S=16, P=9):
  D[n,s]   = att1 . M1[n,:,s] + att2 . M2[n,:,s]
  w[n,p,s] = polycoeffs[p] * D[n,s]^p
  G[n,f]   = sum_{p,s} w[n,p,s] * K1[n,p,s,f]
  E        = G @ weight ; Fden[n] = sum_{p,s} w[n,p,s]*K2[n,p,s]
  out      = E / Fden[:,None]

Sharding: pure data-parallel over nodes, 512 nodes/core, no collectives.

Per-core dataflow (4 blocks x 128 nodes):
  - D via PE matmuls (fp32): lhsT = att replicated to 4 cols, rhs = M1/M2
    laid out [f, (n s)]; result rows live on partitions {0,32,64,96} of one
    PSUM bank (engine partition bases must be 32-aligned).
  - D row -> DRAM bounce -> reload as [n, s] (node-on-partition) layout.
  - Powers/w/Fden all computed in row layout ([128 nodes, ...]); Fden is a
    free-dim reduction; w transposed once per block on the PE into the
    [ps, n] layout the G matmuls need.
  - G computed column-per-node: gt[:, b] = K1[b].T @ w[b] accumulated over
    two ps-chunks: A = ps 16..143 (K=128, bf16, w-dependent) and
    B = ps 0..15 (K=16, constant weight c0 since D^0 == 1).
    bf16 is used for this contraction only (measured end-to-end rel err
    ~2e-3 vs the fp32 pipeline's own ~1e-3 reassociation noise).
  - E = gt.T @ weight in fp32, scaled by 1/Fden, streamed out.
"""

import os
import numpy as np

DEBUG_TAPS = bool(os.environ.get("KERNEL_DEBUG_TAPS"))

N_FULL = 4096
F = 128          # IN_FEAT == OUT_FEAT
S = 16
P = 9
PS = P * S       # 144
NCORES = 8
NS = N_FULL // NCORES   # 512 nodes per core
BLK = 128               # nodes per block
NBLK = NS // BLK        # 4
GRP = 16                # nodes per K1 DMA group
NGRP = BLK // GRP       # 8 groups per block

_BUILT = None


def _build():
    """Build and return the compiled Bass module (cached per process)."""
    global _BUILT
    if _BUILT is not None:
        return _BUILT

    import concourse.bacc as bacc
    import concourse.tile as tile
    import concourse.mybir as mybir
    from concourse import masks

    f32 = mybir.dt.float32
    bf16 = mybir.dt.bfloat16

    nc = bacc.Bacc("TRN2", target_bir_lowering=False, debug=False)

    M1d = nc.dram_tensor("M1", [NS, F, S], f32, kind="ExternalInput").ap()
    M2d = nc.dram_tensor("M2", [NS, F, S], f32, kind="ExternalInput").ap()
    K1d = nc.dram_tensor("K1", [NS, P, S, F], f32, kind="ExternalInput").ap()
    K2d = nc.dram_tensor("K2", [NS, P, S], f32, kind="ExternalInput").ap()
    att1d = nc.dram_tensor("att1", [F], f32, kind="ExternalInput").ap()
    att2d = nc.dram_tensor("att2", [F], f32, kind="ExternalInput").ap()
    wtd = nc.dram_tensor("weight", [F, F], f32, kind="ExternalInput").ap()
    polyd = nc.dram_tensor("polycoeffs", [P], f32, kind="ExternalInput").ap()
    outd = nc.dram_tensor("out", [NS, F], f32, kind="ExternalOutput").ap()
    if DEBUG_TAPS:
        dbg_d = nc.dram_tensor("dbg_d", [NS, S], f32, kind="ExternalOutput").ap()
        dbg_fden = nc.dram_tensor("dbg_fden", [NS, 1], f32, kind="ExternalOutput").ap()
        dbg_gt = nc.dram_tensor("dbg_gt", [NBLK, F, BLK], f32, kind="ExternalOutput").ap()
        dbg_w = nc.dram_tensor("dbg_w", [NS, 128], f32, kind="ExternalOutput").ap()
        dbg_wa = nc.dram_tensor("dbg_wa", [NBLK, 128, BLK], f32, kind="ExternalOutput").ap()
        dbg_k1a = nc.dram_tensor("dbg_k1a", [128, GRP * F], f32, kind="ExternalOutput").ap()
        dbg_k1b = nc.dram_tensor("dbg_k1b", [S, GRP * F], f32, kind="ExternalOutput").ap()

    K1ps = K1d.rearrange("n p s f -> n (p s) f")   # [NS, 144, 128]
    K2ps = K2d.rearrange("n p s -> n (p s)")       # [NS, 144]

    with tile.TileContext(nc) as tc:
        with (
            tc.tile_pool(name="const", bufs=1) as cpool,
            tc.tile_pool(name="m12", bufs=2) as mpool,
            tc.tile_pool(name="k1a", bufs=3) as k1apool,
            tc.tile_pool(name="k1b", bufs=3) as k1bpool,
            tc.tile_pool(name="small", bufs=2) as spool,
            tc.tile_pool(name="pw", bufs=2) as pwpool,
            tc.tile_pool(name="dram", bufs=2, space="DRAM") as dpool,
            tc.tile_pool(name="ps_d", bufs=2, space="PSUM") as psd,
            tc.tile_pool(name="ps_wt", bufs=2, space="PSUM") as pswt,
            tc.tile_pool(name="ps_gt", bufs=2, space="PSUM") as psgt,
            tc.tile_pool(name="ps_e", bufs=1, space="PSUM") as pse,
        ):
            # ---------------- constants ----------------
            w_sb = cpool.tile([F, F], f32)            # weight [f, o]
            nc.sync.dma_start(w_sb[:], wtd[:])

            att1_sb = cpool.tile([F, 1], f32)
            att2_sb = cpool.tile([F, 1], f32)
            nc.sync.dma_start(att1_sb[:], att1d.unsqueeze(1))
            nc.sync.dma_start(att2_sb[:], att2d.unsqueeze(1))
            att1_r4 = cpool.tile([F, 4], f32)
            att2_r4 = cpool.tile([F, 4], f32)
            nc.vector.tensor_copy(att1_r4[:], att1_sb[:, 0:1].broadcast_to([F, 4]))
            nc.vector.tensor_copy(att2_r4[:], att2_sb[:, 0:1].broadcast_to([F, 4]))

            ident = cpool.tile([128, 128], f32)
            masks.make_identity(nc, ident[:])

            # polycoeffs replicated to all partitions: [128, 9]
            poly_row = cpool.tile([1, P], f32)
            nc.sync.dma_start(poly_row[:], polyd.unsqueeze(0))
            ones_row = cpool.tile([1, 128], f32)
            nc.vector.memset(ones_row[:], 1.0)
            poly_ps = pse.tile([128, P], f32)
            nc.tensor.matmul(poly_ps[:], ones_row[:], poly_row[:],
                             start=True, stop=True)
            poly_rep = cpool.tile([128, P], f32)
            nc.vector.tensor_copy(poly_rep[:], poly_ps[:])
            c0col_bf = cpool.tile([S, 1], bf16)
            nc.vector.tensor_copy(c0col_bf[:], poly_rep[0:S, 0:1])

            # ---------------- per-block pipeline ----------------
            for blk in range(NBLK):
                nb = blk * BLK

                # -- DMAs --
                m1t = mpool.tile([F, BLK * S], f32, tag="m1")
                m2t = mpool.tile([F, BLK * S], f32, tag="m2")
                nc.sync.dma_start(m1t[:], M1d[nb:nb + BLK].transpose([1, 0, 2]))
                nc.sync.dma_start(m2t[:], M2d[nb:nb + BLK].transpose([1, 0, 2]))

                k2row = spool.tile([BLK, PS], f32, tag="k2")
                nc.sync.dma_start(k2row[:], K2ps[nb:nb + BLK])

                k1a_g = []
                k1b_g = []
                for g in range(NGRP):
                    n0 = nb + g * GRP
                    ka = k1apool.tile([128, GRP * F], bf16)
                    kb = k1bpool.tile([S, GRP * F], bf16)
                    # cast-during-DMA (SWDGE): fp32 HBM -> bf16 SBUF
                    nc.gpsimd.dma_start(
                        ka[:], K1ps[n0:n0 + GRP, S:PS, :].transpose([1, 0, 2]))
                    nc.gpsimd.dma_start(
                        kb[:], K1ps[n0:n0 + GRP, 0:S, :].transpose([1, 0, 2]))
                    k1a_g.append(ka)
                    k1b_g.append(kb)

                # -- D: PE matmuls, rows on partitions {0,32,64,96} --
                d4_ps = psd.tile([128, 512], f32)
                for c in range(4):
                    o = d4_ps[32 * c:32 * c + 4, :]
                    nc.tensor.matmul(o, att1_r4[:], m1t[:, 512 * c:512 * (c + 1)],
                                     start=True, stop=False,
                                     tile_position=(0, 32 * c))
                    nc.tensor.matmul(o, att2_r4[:], m2t[:, 512 * c:512 * (c + 1)],
                                     start=False, stop=True,
                                     tile_position=(0, 32 * c))

                d_row = spool.tile([1, BLK * S], f32, tag="drow")
                for c in range(4):
                    nc.vector.tensor_copy(d_row[0:1, 512 * c:512 * (c + 1)],
                                          d4_ps[32 * c:32 * c + 1, :])

                # -- bounce through DRAM to flip into [n, s] layout --
                d_dr = dpool.tile([1, BLK * S], f32)
                nc.sync.dma_start(d_dr[:], d_row[:])
                d_ns = spool.tile([BLK, S], f32, tag="dns")
                nc.sync.dma_start(
                    d_ns[:], d_dr[:].rearrange("o (n s) -> (o n) s", s=S))

                # -- powers / w in row layout --
                # w_row col j corresponds to ps = 16 + j  (p = 1..8)
                w_row = spool.tile([BLK, 128], f32, tag="wrow")
                nc.vector.tensor_scalar(w_row[:, 0:S], d_ns[:],
                                        poly_rep[:, 1:2], None,
                                        op0=mybir.AluOpType.mult)
                pcur = d_ns
                for p in range(2, P):
                    pnxt = pwpool.tile([BLK, S], f32, tag="pw")
                    nc.vector.tensor_tensor(out=pnxt[:], in0=pcur[:], in1=d_ns[:],
                                            op=mybir.AluOpType.mult)
                    nc.vector.tensor_scalar(w_row[:, S * (p - 1):S * p], pnxt[:],
                                            poly_rep[:, p:p + 1], None,
                                            op0=mybir.AluOpType.mult)
                    pcur = pnxt

                # -- Fden in row layout: V = w .* K2, reduce over free dim --
                v_row = spool.tile([BLK, PS], f32, tag="vrow")
                nc.vector.tensor_scalar(v_row[:, 0:S], k2row[:, 0:S],
                                        poly_rep[:, 0:1], None,
                                        op0=mybir.AluOpType.mult)
                nc.vector.tensor_tensor(out=v_row[:, S:PS], in0=w_row[:],
                                        in1=k2row[:, S:PS],
                                        op=mybir.AluOpType.mult)
                fden = spool.tile([BLK, 1], f32, tag="fden")
                nc.vector.tensor_reduce(fden[:], v_row[:],
                                        axis=mybir.AxisListType.X,
                                        op=mybir.AluOpType.add)
                rec = spool.tile([BLK, 1], f32, tag="rec")
                nc.vector.reciprocal(rec[:], fden[:])

                # -- transpose w into [ps, n], cast to bf16 --
                wt_ps = pswt.tile([128, 128], f32)
                nc.tensor.transpose(wt_ps[:], w_row[:], ident[:])
                wa_bf = spool.tile([128, BLK], bf16, tag="wabf")
                nc.vector.tensor_copy(wa_bf[:], wt_ps[:])

                # -- G: one PSUM column per node --
                # start=True zeroes the whole 2KB PSUM zero-region (the full
                # bank row), not just the written column — so ONLY the first
                # matmul into the tile may carry it.
                gt_ps = psgt.tile([128, BLK], f32)
                # A-chunk: ps 16..143, K=128 (all in (128,128) tile mode)
                for b in range(BLK):
                    g, col = b // GRP, b % GRP
                    nc.tensor.matmul(gt_ps[:, b:b + 1],
                                     k1a_g[g][:, col * F:(col + 1) * F],
                                     wa_bf[:, b:b + 1],
                                     start=(b == 0), stop=False,
                                     skip_group_check=True)
                # B-chunk: ps 0..15, K=16, constant weight c0 (all (32,128) mode)
                for b in range(BLK):
                    g, col = b // GRP, b % GRP
                    nc.tensor.matmul(gt_ps[:, b:b + 1],
                                     k1b_g[g][:, col * F:(col + 1) * F],
                                     c0col_bf[:],
                                     start=False, stop=(b == BLK - 1),
                                     skip_group_check=True)

                gt_sb = spool.tile([128, BLK], f32, tag="gtsb")
                nc.vector.tensor_copy(gt_sb[:], gt_ps[:])

                if DEBUG_TAPS:
                    nc.sync.dma_start(dbg_d[nb:nb + BLK, :], d_ns[:])
                    nc.sync.dma_start(dbg_fden[nb:nb + BLK, :], fden[:])
                    nc.sync.dma_start(dbg_gt[blk], gt_sb[:])
                    nc.sync.dma_start(dbg_w[nb:nb + BLK, :], w_row[:])
                    wa_f32 = spool.tile([128, BLK], f32, tag="dbgwa")
                    nc.vector.tensor_copy(wa_f32[:], wa_bf[:])
                    nc.sync.dma_start(dbg_wa[blk], wa_f32[:])
                    if blk == 0:
                        k1a_f32 = spool.tile([128, GRP * F], f32, tag="dbgk1a")
                        nc.vector.tensor_copy(k1a_f32[:], k1a_g[0][:])
                        nc.sync.dma_start(dbg_k1a[:], k1a_f32[:])
                        k1b_f32 = spool.tile([S, GRP * F], f32, tag="dbgk1b")
                        nc.vector.tensor_copy(k1b_f32[:], k1b_g[0][:])
                        nc.sync.dma_start(dbg_k1b[:], k1b_f32[:])

                # -- E = gt.T @ weight (fp32), scale rows by 1/Fden --
                e_ps = pse.tile([BLK, F], f32)
                nc.tensor.matmul(e_ps[:], gt_sb[:], w_sb[:], start=True, stop=True)
                out_sb = spool.tile([BLK, F], f32, tag="outsb")
                nc.vector.tensor_scalar(out_sb[:], e_ps[:], rec[:], None,
                                        op0=mybir.AluOpType.mult)
                nc.sync.dma_start(outd[nb:nb + BLK, :], out_sb[:])

    nc.compile()
    _BUILT = nc
    return nc


def _run_sharded(inputs, trace=False, trace_kwargs=None):
    """Shard inputs over 8 cores, run, gather. Returns (out, BassKernelResults)."""
    from concourse.bass_utils import run_bass_kernel_spmd

    M1 = np.ascontiguousarray(np.asarray(inputs["M1"], dtype=np.float32))
    M2 = np.ascontiguousarray(np.asarray(inputs["M2"], dtype=np.float32))
    K1 = np.ascontiguousarray(np.asarray(inputs["K1"], dtype=np.float32))
    K2 = np.ascontiguousarray(np.asarray(inputs["K2"], dtype=np.float32))
    att1 = np.ascontiguousarray(np.asarray(inputs["att1"], dtype=np.float32))
    att2 = np.ascontiguousarray(np.asarray(inputs["att2"], dtype=np.float32))
    weight = np.ascontiguousarray(np.asarray(inputs["weight"], dtype=np.float32))
    poly = np.ascontiguousarray(np.asarray(inputs["polycoeffs"], dtype=np.float32))

    nc = _build()
    in_maps = []
    for c in range(NCORES):
        lo, hi = c * NS, (c + 1) * NS
        in_maps.append({
            "M1": M1[lo:hi], "M2": M2[lo:hi],
            "K1": K1[lo:hi], "K2": K2[lo:hi],
            "att1": att1, "att2": att2, "weight": weight,
            "polycoeffs": poly,
        })
    kwargs = {}
    if trace:
        kwargs["trace"] = True
        if trace_kwargs:
            kwargs.update(trace_kwargs)
    res = run_bass_kernel_spmd(nc, in_maps, core_ids=list(range(NCORES)), **kwargs)
    out = np.concatenate([res.results[c]["out"] for c in range(NCORES)], axis=0)
    return out, res


def kernel(**inputs):
    out, _ = _run_sharded(inputs, trace=False)
    return out


# revision 18
# speedup vs baseline: 1.0351x; 1.0351x over previous
"""FedGATConv forward kernel for Trainium2 (Bass/Tile), 8-core data-parallel.

Computation per node n (N=4096, F=128, S=16, P=9):
  D[n,s]   = att1 . M1[n,:,s] + att2 . M2[n,:,s]
  w[n,p,s] = polycoeffs[p] * D[n,s]^p
  G[n,f]   = sum_{p,s} w[n,p,s] * K1[n,p,s,f]
  E        = G @ weight ; Fden[n] = sum_{p,s} w[n,p,s]*K2[n,p,s]
  out      = E / Fden[:,None]

Sharding: pure data-parallel over nodes, 512 nodes/core, no collectives.

Per-core dataflow (4 blocks x 128 nodes):
  - D via PE matmuls (fp32): lhsT = att replicated to 4 cols, rhs = M1/M2
    laid out [f, (n s)]; result rows live on partitions {0,32,64,96} of one
    PSUM bank (engine partition bases must be 32-aligned).
  - D row -> DRAM bounce -> reload as [n, s] (node-on-partition) layout.
  - Powers/w/Fden all computed in row layout ([128 nodes, ...]); Fden is a
    free-dim reduction; w transposed once per block on the PE into the
    [ps, n] layout the G matmuls need.
  - G computed column-per-node: gt[:, b] = K1[b].T @ w[b] accumulated over
    two ps-chunks: A = ps 16..143 (K=128, bf16, w-dependent) and
    B = ps 0..15 (K=16, constant weight c0 since D^0 == 1).
    bf16 is used for this contraction only (measured end-to-end rel err
    ~2e-3 vs the fp32 pipeline's own ~1e-3 reassociation noise).
  - E = gt.T @ weight in fp32, scaled by 1/Fden, streamed out.
"""

import os
import numpy as np

DEBUG_TAPS = bool(os.environ.get("KERNEL_DEBUG_TAPS"))

N_FULL = 4096
F = 128          # IN_FEAT == OUT_FEAT
S = 16
P = 9
PS = P * S       # 144
NCORES = 8
NS = N_FULL // NCORES   # 512 nodes per core
BLK = 128               # nodes per block
NBLK = NS // BLK        # 4
GRP = 16                # nodes per K1 DMA group
NGRP = BLK // GRP       # 8 groups per block

_BUILT = None


def _build():
    """Build and return the compiled Bass module (cached per process)."""
    global _BUILT
    if _BUILT is not None:
        return _BUILT

    import concourse.bacc as bacc
    import concourse.tile as tile
    import concourse.mybir as mybir
    from concourse import masks

    f32 = mybir.dt.float32
    bf16 = mybir.dt.bfloat16

    nc = bacc.Bacc("TRN2", target_bir_lowering=False, debug=False)

    M1d = nc.dram_tensor("M1", [NS, F, S], f32, kind="ExternalInput").ap()
    M2d = nc.dram_tensor("M2", [NS, F, S], f32, kind="ExternalInput").ap()
    K1d = nc.dram_tensor("K1", [NS, P, S, F], f32, kind="ExternalInput").ap()
    K2d = nc.dram_tensor("K2", [NS, P, S], f32, kind="ExternalInput").ap()
    att1d = nc.dram_tensor("att1", [F], f32, kind="ExternalInput").ap()
    att2d = nc.dram_tensor("att2", [F], f32, kind="ExternalInput").ap()
    wtd = nc.dram_tensor("weight", [F, F], f32, kind="ExternalInput").ap()
    polyd = nc.dram_tensor("polycoeffs", [P], f32, kind="ExternalInput").ap()
    outd = nc.dram_tensor("out", [NS, F], f32, kind="ExternalOutput").ap()
    if DEBUG_TAPS:
        dbg_d = nc.dram_tensor("dbg_d", [NS, S], f32, kind="ExternalOutput").ap()
        dbg_fden = nc.dram_tensor("dbg_fden", [NS, 1], f32, kind="ExternalOutput").ap()
        dbg_gt = nc.dram_tensor("dbg_gt", [NBLK, F, BLK], f32, kind="ExternalOutput").ap()
        dbg_w = nc.dram_tensor("dbg_w", [NS, 128], f32, kind="ExternalOutput").ap()
        dbg_wa = nc.dram_tensor("dbg_wa", [NBLK, 128, BLK], f32, kind="ExternalOutput").ap()
        dbg_k1a = nc.dram_tensor("dbg_k1a", [128, GRP * F], f32, kind="ExternalOutput").ap()
        dbg_k1b = nc.dram_tensor("dbg_k1b", [S, GRP * F], f32, kind="ExternalOutput").ap()

    K1ps = K1d.rearrange("n p s f -> n (p s) f")   # [NS, 144, 128]
    K2ps = K2d.rearrange("n p s -> n (p s)")       # [NS, 144]

    with tile.TileContext(nc) as tc:
        with (
            tc.tile_pool(name="const", bufs=1) as cpool,
            tc.tile_pool(name="m12", bufs=2) as mpool,
            tc.tile_pool(name="k1a", bufs=3) as k1apool,
            tc.tile_pool(name="k1b", bufs=3) as k1bpool,
            tc.tile_pool(name="small", bufs=2) as spool,
            tc.tile_pool(name="pw", bufs=2) as pwpool,
            tc.tile_pool(name="ps_wt", bufs=2, space="PSUM") as pswt,
            tc.tile_pool(name="ps_gt", bufs=2, space="PSUM") as psgt,
            tc.tile_pool(name="ps_e", bufs=1, space="PSUM") as pse,
        ):
            # ---------------- constants ----------------
            w_sb = cpool.tile([F, F], f32)            # weight [f, o]
            nc.sync.dma_start(w_sb[:], wtd[:])

            ident = cpool.tile([128, 128], f32)
            masks.make_identity(nc, ident[:])

            ones_row = cpool.tile([1, 128], f32)
            nc.vector.memset(ones_row[:], 1.0)

            # replicate row vectors to all 128 partitions via K=1 matmuls
            # polycoeffs -> [128, 9]; att1/att2 -> [128, 128]
            poly_row = cpool.tile([1, P], f32)
            nc.sync.dma_start(poly_row[:], polyd.unsqueeze(0))
            att1_row = cpool.tile([1, F], f32)
            att2_row = cpool.tile([1, F], f32)
            nc.sync.dma_start(att1_row[:], att1d.unsqueeze(0))
            nc.sync.dma_start(att2_row[:], att2d.unsqueeze(0))

            poly_ps = pse.tile([128, P], f32, tag="polyps")
            nc.tensor.matmul(poly_ps[:], ones_row[:], poly_row[:],
                             start=True, stop=True)
            poly_rep = cpool.tile([128, P], f32)
            nc.vector.tensor_copy(poly_rep[:], poly_ps[:])
            c0col_bf = cpool.tile([S, 1], bf16)
            nc.vector.tensor_copy(c0col_bf[:], poly_rep[0:S, 0:1])

            att1_bc = cpool.tile([128, F], f32)
            att2_bc = cpool.tile([128, F], f32)
            for row, bc in ((att1_row, att1_bc), (att2_row, att2_bc)):
                ps_t = pse.tile([128, F], f32, tag="attps")
                nc.tensor.matmul(ps_t[:], ones_row[:], row[:], start=True, stop=True)
                nc.vector.tensor_copy(bc[:], ps_t[:])

            # ---------------- per-block pipeline ----------------
            for blk in range(NBLK):
                nb = blk * BLK

                # -- DMAs --
                # natural layout: per partition = one node, 8KB contiguous
                m1n = mpool.tile([BLK, F * S], f32, tag="m1")
                m2n = mpool.tile([BLK, F * S], f32, tag="m2")
                nc.sync.dma_start(m1n[:], M1d[nb:nb + BLK].rearrange("n f s -> n (f s)"))
                nc.sync.dma_start(m2n[:], M2d[nb:nb + BLK].rearrange("n f s -> n (f s)"))

                k2row = spool.tile([BLK, PS], f32, tag="k2")
                nc.sync.dma_start(k2row[:], K2ps[nb:nb + BLK])

                k1a_g = []
                k1b_g = []
                for g in range(NGRP):
                    n0 = nb + g * GRP
                    ka = k1apool.tile([128, GRP * F], bf16)
                    kb = k1bpool.tile([S, GRP * F], bf16)
                    # cast-during-DMA (SWDGE): fp32 HBM -> bf16 SBUF
                    nc.gpsimd.dma_start(
                        ka[:], K1ps[n0:n0 + GRP, S:PS, :].transpose([1, 0, 2]))
                    nc.gpsimd.dma_start(
                        kb[:], K1ps[n0:n0 + GRP, 0:S, :].transpose([1, 0, 2]))
                    k1a_g.append(ka)
                    k1b_g.append(kb)

                # -- D on DVE, directly in [n, s] layout --
                # D[n,s] = sum_f att1[f]*M1[n,f,s] + att2[f]*M2[n,f,s];
                # m1n col = f*16+s, so the f-slice for fixed s is [:, s::16].
                d_ns = spool.tile([BLK, S], f32, tag="dns")
                d_tmp = spool.tile([BLK, S], f32, tag="dtmp")
                scr = spool.tile([BLK, F], f32, tag="dscr")
                for s in range(S):
                    nc.vector.tensor_tensor(out=scr[:], in0=m1n[:, s::S],
                                            in1=att1_bc[:], op=mybir.AluOpType.mult)
                    nc.vector.tensor_reduce(d_tmp[:, s:s + 1], scr[:],
                                            axis=mybir.AxisListType.X,
                                            op=mybir.AluOpType.add)
                    nc.vector.tensor_tensor(out=scr[:], in0=m2n[:, s::S],
                                            in1=att2_bc[:], op=mybir.AluOpType.mult)
                    nc.vector.tensor_reduce(d_ns[:, s:s + 1], scr[:],
                                            axis=mybir.AxisListType.X,
                                            op=mybir.AluOpType.add)
                    nc.vector.tensor_tensor(out=d_ns[:, s:s + 1], in0=d_ns[:, s:s + 1],
                                            in1=d_tmp[:, s:s + 1], op=mybir.AluOpType.add)

                # -- powers / w in row layout --
                # w_row col j corresponds to ps = 16 + j  (p = 1..8)
                w_row = spool.tile([BLK, 128], f32, tag="wrow")
                nc.vector.tensor_scalar(w_row[:, 0:S], d_ns[:],
                                        poly_rep[:, 1:2], None,
                                        op0=mybir.AluOpType.mult)
                pcur = d_ns
                for p in range(2, P):
                    pnxt = pwpool.tile([BLK, S], f32, tag="pw")
                    nc.vector.tensor_tensor(out=pnxt[:], in0=pcur[:], in1=d_ns[:],
                                            op=mybir.AluOpType.mult)
                    nc.vector.tensor_scalar(w_row[:, S * (p - 1):S * p], pnxt[:],
                                            poly_rep[:, p:p + 1], None,
                                            op0=mybir.AluOpType.mult)
                    pcur = pnxt

                # -- Fden in row layout: V = w .* K2, reduce over free dim --
                v_row = spool.tile([BLK, PS], f32, tag="vrow")
                nc.vector.tensor_scalar(v_row[:, 0:S], k2row[:, 0:S],
                                        poly_rep[:, 0:1], None,
                                        op0=mybir.AluOpType.mult)
                nc.vector.tensor_tensor(out=v_row[:, S:PS], in0=w_row[:],
                                        in1=k2row[:, S:PS],
                                        op=mybir.AluOpType.mult)
                fden = spool.tile([BLK, 1], f32, tag="fden")
                nc.vector.tensor_reduce(fden[:], v_row[:],
                                        axis=mybir.AxisListType.X,
                                        op=mybir.AluOpType.add)
                rec = spool.tile([BLK, 1], f32, tag="rec")
                nc.vector.reciprocal(rec[:], fden[:])

                # -- transpose w into [ps, n], cast to bf16 --
                wt_ps = pswt.tile([128, 128], f32)
                nc.tensor.transpose(wt_ps[:], w_row[:], ident[:])
                wa_bf = spool.tile([128, BLK], bf16, tag="wabf")
                nc.vector.tensor_copy(wa_bf[:], wt_ps[:])

                # -- G: one PSUM column per node --
                # start=True zeroes the whole 2KB PSUM zero-region (the full
                # bank row), not just the written column — so ONLY the first
                # matmul into the tile may carry it.
                gt_ps = psgt.tile([128, BLK], f32)
                # A-chunk: ps 16..143, K=128 (all in (128,128) tile mode)
                for b in range(BLK):
                    g, col = b // GRP, b % GRP
                    nc.tensor.matmul(gt_ps[:, b:b + 1],
                                     k1a_g[g][:, col * F:(col + 1) * F],
                                     wa_bf[:, b:b + 1],
                                     start=(b == 0), stop=False,
                                     skip_group_check=True)
                # B-chunk: ps 0..15, K=16, constant weight c0 (all (32,128) mode)
                for b in range(BLK):
                    g, col = b // GRP, b % GRP
                    nc.tensor.matmul(gt_ps[:, b:b + 1],
                                     k1b_g[g][:, col * F:(col + 1) * F],
                                     c0col_bf[:],
                                     start=False, stop=(b == BLK - 1),
                                     skip_group_check=True)

                gt_sb = spool.tile([128, BLK], f32, tag="gtsb")
                nc.vector.tensor_copy(gt_sb[:], gt_ps[:])

                if DEBUG_TAPS:
                    nc.sync.dma_start(dbg_d[nb:nb + BLK, :], d_ns[:])
                    nc.sync.dma_start(dbg_fden[nb:nb + BLK, :], fden[:])
                    nc.sync.dma_start(dbg_gt[blk], gt_sb[:])
                    nc.sync.dma_start(dbg_w[nb:nb + BLK, :], w_row[:])
                    wa_f32 = spool.tile([128, BLK], f32, tag="dbgwa")
                    nc.vector.tensor_copy(wa_f32[:], wa_bf[:])
                    nc.sync.dma_start(dbg_wa[blk], wa_f32[:])
                    if blk == 0:
                        k1a_f32 = spool.tile([128, GRP * F], f32, tag="dbgk1a")
                        nc.vector.tensor_copy(k1a_f32[:], k1a_g[0][:])
                        nc.sync.dma_start(dbg_k1a[:], k1a_f32[:])
                        k1b_f32 = spool.tile([S, GRP * F], f32, tag="dbgk1b")
                        nc.vector.tensor_copy(k1b_f32[:], k1b_g[0][:])
                        nc.sync.dma_start(dbg_k1b[:], k1b_f32[:])

                # -- E = gt.T @ weight (fp32), scale rows by 1/Fden --
                e_ps = pse.tile([BLK, F], f32)
                nc.tensor.matmul(e_ps[:], gt_sb[:], w_sb[:], start=True, stop=True)
                out_sb = spool.tile([BLK, F], f32, tag="outsb")
                nc.vector.tensor_scalar(out_sb[:], e_ps[:], rec[:], None,
                                        op0=mybir.AluOpType.mult)
                nc.sync.dma_start(outd[nb:nb + BLK, :], out_sb[:])

    nc.compile()
    _BUILT = nc
    return nc


def _run_sharded(inputs, trace=False, trace_kwargs=None):
    """Shard inputs over 8 cores, run, gather. Returns (out, BassKernelResults)."""
    from concourse.bass_utils import run_bass_kernel_spmd

    M1 = np.ascontiguousarray(np.asarray(inputs["M1"], dtype=np.float32))
    M2 = np.ascontiguousarray(np.asarray(inputs["M2"], dtype=np.float32))
    K1 = np.ascontiguousarray(np.asarray(inputs["K1"], dtype=np.float32))
    K2 = np.ascontiguousarray(np.asarray(inputs["K2"], dtype=np.float32))
    att1 = np.ascontiguousarray(np.asarray(inputs["att1"], dtype=np.float32))
    att2 = np.ascontiguousarray(np.asarray(inputs["att2"], dtype=np.float32))
    weight = np.ascontiguousarray(np.asarray(inputs["weight"], dtype=np.float32))
    poly = np.ascontiguousarray(np.asarray(inputs["polycoeffs"], dtype=np.float32))

    nc = _build()
    in_maps = []
    for c in range(NCORES):
        lo, hi = c * NS, (c + 1) * NS
        in_maps.append({
            "M1": M1[lo:hi], "M2": M2[lo:hi],
            "K1": K1[lo:hi], "K2": K2[lo:hi],
            "att1": att1, "att2": att2, "weight": weight,
            "polycoeffs": poly,
        })
    kwargs = {}
    if trace:
        kwargs["trace"] = True
        if trace_kwargs:
            kwargs.update(trace_kwargs)
    res = run_bass_kernel_spmd(nc, in_maps, core_ids=list(range(NCORES)), **kwargs)
    out = np.concatenate([res.results[c]["out"] for c in range(NCORES)], axis=0)
    return out, res


def kernel(**inputs):
    out, _ = _run_sharded(inputs, trace=False)
    return out


# revision 21
# speedup vs baseline: 1.5338x; 1.4818x over previous
"""FedGATConv forward kernel for Trainium2 (Bass/Tile), 8-core data-parallel.

Computation per node n (N=4096, F=128, S=16, P=9):
  D[n,s]   = att1 . M1[n,:,s] + att2 . M2[n,:,s]
  w[n,p,s] = polycoeffs[p] * D[n,s]^p
  G[n,f]   = sum_{p,s} w[n,p,s] * K1[n,p,s,f]
  E        = G @ weight ; Fden[n] = sum_{p,s} w[n,p,s]*K2[n,p,s]
  out      = E / Fden[:,None]

Sharding: pure data-parallel over nodes, 512 nodes/core, no collectives.

Per-core dataflow (4 blocks x 128 nodes):
  - D via PE matmuls (fp32): lhsT = att replicated to 4 cols, rhs = M1/M2
    laid out [f, (n s)]; result rows live on partitions {0,32,64,96} of one
    PSUM bank (engine partition bases must be 32-aligned).
  - D row -> DRAM bounce -> reload as [n, s] (node-on-partition) layout.
  - Powers/w/Fden all computed in row layout ([128 nodes, ...]); Fden is a
    free-dim reduction; w transposed once per block on the PE into the
    [ps, n] layout the G matmuls need.
  - G computed column-per-node: gt[:, b] = K1[b].T @ w[b] accumulated over
    two ps-chunks: A = ps 16..143 (K=128, bf16, w-dependent) and
    B = ps 0..15 (K=16, constant weight c0 since D^0 == 1).
    bf16 is used for this contraction only (measured end-to-end rel err
    ~2e-3 vs the fp32 pipeline's own ~1e-3 reassociation noise).
  - E = gt.T @ weight in fp32, scaled by 1/Fden, streamed out.
"""

import os
import numpy as np

DEBUG_TAPS = bool(os.environ.get("KERNEL_DEBUG_TAPS"))

N_FULL = 4096
F = 128          # IN_FEAT == OUT_FEAT
S = 16
P = 9
PS = P * S       # 144
NCORES = 8
NS = N_FULL // NCORES   # 512 nodes per core
BLK = 128               # nodes per block
NBLK = NS // BLK        # 4
GRP = 32                # nodes per K1 DMA group
NGRP = BLK // GRP       # 4 groups per block

_BUILT = None


def _build():
    """Build and return the compiled Bass module (cached per process)."""
    global _BUILT
    if _BUILT is not None:
        return _BUILT

    import concourse.bacc as bacc
    import concourse.tile as tile
    import concourse.mybir as mybir
    from concourse import masks

    f32 = mybir.dt.float32
    bf16 = mybir.dt.bfloat16

    nc = bacc.Bacc("TRN2", target_bir_lowering=False, debug=False)

    M1d = nc.dram_tensor("M1", [NS, F, S], f32, kind="ExternalInput").ap()
    M2d = nc.dram_tensor("M2", [NS, F, S], f32, kind="ExternalInput").ap()
    K1d = nc.dram_tensor("K1", [NS, P, S, F], f32, kind="ExternalInput").ap()
    K2d = nc.dram_tensor("K2", [NS, P, S], f32, kind="ExternalInput").ap()
    att1d = nc.dram_tensor("att1", [F], f32, kind="ExternalInput").ap()
    att2d = nc.dram_tensor("att2", [F], f32, kind="ExternalInput").ap()
    wtd = nc.dram_tensor("weight", [F, F], f32, kind="ExternalInput").ap()
    polyd = nc.dram_tensor("polycoeffs", [P], f32, kind="ExternalInput").ap()
    outd = nc.dram_tensor("out", [NS, F], f32, kind="ExternalOutput").ap()
    if DEBUG_TAPS:
        dbg_d = nc.dram_tensor("dbg_d", [NS, S], f32, kind="ExternalOutput").ap()
        dbg_fden = nc.dram_tensor("dbg_fden", [NS, 1], f32, kind="ExternalOutput").ap()
        dbg_gt = nc.dram_tensor("dbg_gt", [NBLK, F, BLK], f32, kind="ExternalOutput").ap()
        dbg_w = nc.dram_tensor("dbg_w", [NS, 128], f32, kind="ExternalOutput").ap()
        dbg_wa = nc.dram_tensor("dbg_wa", [NBLK, 128, BLK], f32, kind="ExternalOutput").ap()
        dbg_k1a = nc.dram_tensor("dbg_k1a", [128, GRP * F], f32, kind="ExternalOutput").ap()
        dbg_k1b = nc.dram_tensor("dbg_k1b", [S, GRP * F], f32, kind="ExternalOutput").ap()

    K1ps = K1d.rearrange("n p s f -> n (p s) f")   # [NS, 144, 128]
    K2ps = K2d.rearrange("n p s -> n (p s)")       # [NS, 144]

    with tile.TileContext(nc) as tc:
        with (
            tc.tile_pool(name="const", bufs=1) as cpool,
            tc.tile_pool(name="m12", bufs=2) as mpool,
            tc.tile_pool(name="k1a", bufs=6) as k1apool,
            tc.tile_pool(name="k1b", bufs=6) as k1bpool,
            tc.tile_pool(name="small", bufs=2) as spool,
            tc.tile_pool(name="pw", bufs=2) as pwpool,
            tc.tile_pool(name="ps_wt", bufs=2, space="PSUM") as pswt,
            tc.tile_pool(name="ps_gt", bufs=2, space="PSUM") as psgt,
            tc.tile_pool(name="ps_e", bufs=1, space="PSUM") as pse,
        ):
            # ---------------- constants ----------------
            w_sb = cpool.tile([F, F], f32)            # weight [f, o]
            nc.sync.dma_start(w_sb[:], wtd[:])

            ident = cpool.tile([128, 128], f32)
            masks.make_identity(nc, ident[:])

            ones_row = cpool.tile([1, 128], f32)
            nc.vector.memset(ones_row[:], 1.0)

            # replicate row vectors to all 128 partitions via K=1 matmuls
            # polycoeffs -> [128, 9]; att1/att2 -> [128, 128]
            poly_row = cpool.tile([1, P], f32)
            nc.sync.dma_start(poly_row[:], polyd.unsqueeze(0))
            att1_row = cpool.tile([1, F], f32)
            att2_row = cpool.tile([1, F], f32)
            nc.sync.dma_start(att1_row[:], att1d.unsqueeze(0))
            nc.sync.dma_start(att2_row[:], att2d.unsqueeze(0))

            poly_ps = pse.tile([128, P], f32, tag="polyps")
            nc.tensor.matmul(poly_ps[:], ones_row[:], poly_row[:],
                             start=True, stop=True)
            poly_rep = cpool.tile([128, P], f32)
            nc.vector.tensor_copy(poly_rep[:], poly_ps[:])
            c0col_bf = cpool.tile([S, 1], bf16)
            nc.vector.tensor_copy(c0col_bf[:], poly_rep[0:S, 0:1])

            att1_bc = cpool.tile([128, F], f32)
            att2_bc = cpool.tile([128, F], f32)
            for row, bc in ((att1_row, att1_bc), (att2_row, att2_bc)):
                ps_t = pse.tile([128, F], f32, tag="attps")
                nc.tensor.matmul(ps_t[:], ones_row[:], row[:], start=True, stop=True)
                nc.vector.tensor_copy(bc[:], ps_t[:])

            # ---------------- per-block pipeline ----------------
            for blk in range(NBLK):
                nb = blk * BLK

                # -- DMAs --
                # natural layout: per partition = one node, 8KB contiguous
                m1n = mpool.tile([BLK, F * S], f32, tag="m1")
                m2n = mpool.tile([BLK, F * S], f32, tag="m2")
                nc.sync.dma_start(m1n[:], M1d[nb:nb + BLK].rearrange("n f s -> n (f s)"))
                nc.sync.dma_start(m2n[:], M2d[nb:nb + BLK].rearrange("n f s -> n (f s)"))

                k2row = spool.tile([BLK, PS], f32, tag="k2")
                nc.sync.dma_start(k2row[:], K2ps[nb:nb + BLK])

                k1a_g = []
                k1b_g = []
                for g in range(NGRP):
                    n0 = nb + g * GRP
                    ka = k1apool.tile([128, GRP * F], bf16)
                    kb = k1bpool.tile([S, GRP * F], bf16)
                    # cast-during-DMA (SWDGE): fp32 HBM -> bf16 SBUF
                    nc.gpsimd.dma_start(
                        ka[:], K1ps[n0:n0 + GRP, S:PS, :].transpose([1, 0, 2]))
                    nc.gpsimd.dma_start(
                        kb[:], K1ps[n0:n0 + GRP, 0:S, :].transpose([1, 0, 2]))
                    k1a_g.append(ka)
                    k1b_g.append(kb)

                # -- D on DVE, directly in [n, s] layout --
                # D[n,s] = sum_f att1[f]*M1[n,f,s] + att2[f]*M2[n,f,s].
                # m1n col = f*16+s; att broadcast-expanded along s (step-0 AP);
                # single strided reduce over f via a [n, s, f] view.
                att1_x = att1_bc[:].unsqueeze(2).broadcast_to([BLK, F, S])
                att2_x = att2_bc[:].unsqueeze(2).broadcast_to([BLK, F, S])
                m1v = m1n[:].rearrange("n (f s) -> n f s", s=S)
                m2v = m2n[:].rearrange("n (f s) -> n f s", s=S)
                prod = mpool.tile([BLK, F * S], f32, tag="prod")
                prodv = prod[:].rearrange("n (f s) -> n f s", s=S)
                prodr = prod[:].rearrange("n (f s) -> n s f", s=S)
                d_ns = spool.tile([BLK, S], f32, tag="dns")
                d_tmp = spool.tile([BLK, S], f32, tag="dtmp")
                nc.vector.tensor_tensor(out=prodv, in0=m1v, in1=att1_x,
                                        op=mybir.AluOpType.mult)
                nc.vector.tensor_reduce(d_tmp[:], prodr,
                                        axis=mybir.AxisListType.X,
                                        op=mybir.AluOpType.add)
                nc.vector.tensor_tensor(out=prodv, in0=m2v, in1=att2_x,
                                        op=mybir.AluOpType.mult)
                nc.vector.tensor_reduce(d_ns[:], prodr,
                                        axis=mybir.AxisListType.X,
                                        op=mybir.AluOpType.add)
                nc.vector.tensor_tensor(out=d_ns[:], in0=d_ns[:], in1=d_tmp[:],
                                        op=mybir.AluOpType.add)

                # -- powers / w in row layout --
                # w_row col j corresponds to ps = 16 + j  (p = 1..8)
                w_row = spool.tile([BLK, 128], f32, tag="wrow")
                nc.vector.tensor_scalar(w_row[:, 0:S], d_ns[:],
                                        poly_rep[:, 1:2], None,
                                        op0=mybir.AluOpType.mult)
                pcur = d_ns
                for p in range(2, P):
                    pnxt = pwpool.tile([BLK, S], f32, tag="pw")
                    nc.vector.tensor_tensor(out=pnxt[:], in0=pcur[:], in1=d_ns[:],
                                            op=mybir.AluOpType.mult)
                    nc.vector.tensor_scalar(w_row[:, S * (p - 1):S * p], pnxt[:],
                                            poly_rep[:, p:p + 1], None,
                                            op0=mybir.AluOpType.mult)
                    pcur = pnxt

                # -- Fden in row layout: V = w .* K2, reduce over free dim --
                v_row = spool.tile([BLK, PS], f32, tag="vrow")
                nc.vector.tensor_scalar(v_row[:, 0:S], k2row[:, 0:S],
                                        poly_rep[:, 0:1], None,
                                        op0=mybir.AluOpType.mult)
                nc.vector.tensor_tensor(out=v_row[:, S:PS], in0=w_row[:],
                                        in1=k2row[:, S:PS],
                                        op=mybir.AluOpType.mult)
                fden = spool.tile([BLK, 1], f32, tag="fden")
                nc.vector.tensor_reduce(fden[:], v_row[:],
                                        axis=mybir.AxisListType.X,
                                        op=mybir.AluOpType.add)
                rec = spool.tile([BLK, 1], f32, tag="rec")
                nc.vector.reciprocal(rec[:], fden[:])

                # -- transpose w into [ps, n], cast to bf16 --
                wt_ps = pswt.tile([128, 128], f32)
                nc.tensor.transpose(wt_ps[:], w_row[:], ident[:])
                wa_bf = spool.tile([128, BLK], bf16, tag="wabf")
                nc.vector.tensor_copy(wa_bf[:], wt_ps[:])

                # -- G: one PSUM column per node --
                # start=True zeroes the whole 2KB PSUM zero-region (the full
                # bank row), not just the written column — so ONLY the first
                # matmul into the tile may carry it.
                gt_ps = psgt.tile([128, BLK], f32)
                # A-chunk: ps 16..143, K=128 (all in (128,128) tile mode)
                for b in range(BLK):
                    g, col = b // GRP, b % GRP
                    nc.tensor.matmul(gt_ps[:, b:b + 1],
                                     k1a_g[g][:, col * F:(col + 1) * F],
                                     wa_bf[:, b:b + 1],
                                     start=(b == 0), stop=False,
                                     skip_group_check=True)
                # B-chunk: ps 0..15, K=16, constant weight c0 (all (32,128) mode)
                for b in range(BLK):
                    g, col = b // GRP, b % GRP
                    nc.tensor.matmul(gt_ps[:, b:b + 1],
                                     k1b_g[g][:, col * F:(col + 1) * F],
                                     c0col_bf[:],
                                     start=False, stop=(b == BLK - 1),
                                     skip_group_check=True)

                gt_sb = spool.tile([128, BLK], f32, tag="gtsb")
                nc.vector.tensor_copy(gt_sb[:], gt_ps[:])

                if DEBUG_TAPS:
                    nc.sync.dma_start(dbg_d[nb:nb + BLK, :], d_ns[:])
                    nc.sync.dma_start(dbg_fden[nb:nb + BLK, :], fden[:])
                    nc.sync.dma_start(dbg_gt[blk], gt_sb[:])
                    nc.sync.dma_start(dbg_w[nb:nb + BLK, :], w_row[:])
                    wa_f32 = spool.tile([128, BLK], f32, tag="dbgwa")
                    nc.vector.tensor_copy(wa_f32[:], wa_bf[:])
                    nc.sync.dma_start(dbg_wa[blk], wa_f32[:])
                    if blk == 0:
                        k1a_f32 = spool.tile([128, GRP * F], f32, tag="dbgk1a")
                        nc.vector.tensor_copy(k1a_f32[:], k1a_g[0][:])
                        nc.sync.dma_start(dbg_k1a[:], k1a_f32[:])
                        k1b_f32 = spool.tile([S, GRP * F], f32, tag="dbgk1b")
                        nc.vector.tensor_copy(k1b_f32[:], k1b_g[0][:])
                        nc.sync.dma_start(dbg_k1b[:], k1b_f32[:])

                # -- E = gt.T @ weight (fp32), scale rows by 1/Fden --
                e_ps = pse.tile([BLK, F], f32)
                nc.tensor.matmul(e_ps[:], gt_sb[:], w_sb[:], start=True, stop=True)
                out_sb = spool.tile([BLK, F], f32, tag="outsb")
                nc.vector.tensor_scalar(out_sb[:], e_ps[:], rec[:], None,
                                        op0=mybir.AluOpType.mult)
                nc.sync.dma_start(outd[nb:nb + BLK, :], out_sb[:])

    nc.compile()
    _BUILT = nc
    return nc


def _run_sharded(inputs, trace=False, trace_kwargs=None):
    """Shard inputs over 8 cores, run, gather. Returns (out, BassKernelResults)."""
    from concourse.bass_utils import run_bass_kernel_spmd

    M1 = np.ascontiguousarray(np.asarray(inputs["M1"], dtype=np.float32))
    M2 = np.ascontiguousarray(np.asarray(inputs["M2"], dtype=np.float32))
    K1 = np.ascontiguousarray(np.asarray(inputs["K1"], dtype=np.float32))
    K2 = np.ascontiguousarray(np.asarray(inputs["K2"], dtype=np.float32))
    att1 = np.ascontiguousarray(np.asarray(inputs["att1"], dtype=np.float32))
    att2 = np.ascontiguousarray(np.asarray(inputs["att2"], dtype=np.float32))
    weight = np.ascontiguousarray(np.asarray(inputs["weight"], dtype=np.float32))
    poly = np.ascontiguousarray(np.asarray(inputs["polycoeffs"], dtype=np.float32))

    nc = _build()
    in_maps = []
    for c in range(NCORES):
        lo, hi = c * NS, (c + 1) * NS
        in_maps.append({
            "M1": M1[lo:hi], "M2": M2[lo:hi],
            "K1": K1[lo:hi], "K2": K2[lo:hi],
            "att1": att1, "att2": att2, "weight": weight,
            "polycoeffs": poly,
        })
    kwargs = {}
    if trace:
        kwargs["trace"] = True
        if trace_kwargs:
            kwargs.update(trace_kwargs)
    res = run_bass_kernel_spmd(nc, in_maps, core_ids=list(range(NCORES)), **kwargs)
    out = np.concatenate([res.results[c]["out"] for c in range(NCORES)], axis=0)
    return out, res


def kernel(**inputs):
    out, _ = _run_sharded(inputs, trace=False)
    return out
